# revision 11
# baseline (speedup 1.0000x reference)
"""ChebNet (K=3, 4 layers, H=200) on 8 TRN2 NeuronCores.

Strategy: data-parallel over graphs (32 graphs / core, contiguous node
ranges since batch is sorted). The sparse propagation L_hat@h is done as
  gather h[src] rows (dma_gather from a replicated copy in HBM)
  -> segmented matmul against an on-chip-built selection matrix M
     (M[e, d] = w[e] * (dst_local[e] == d)) accumulating per dst tile.
Replication of h across cores between propagations via AllGather.
The Chebyshev recurrence is refactored to avoid the *2/-Tx0 fixups:
  out = Tx0@(W0-W2) + Tx1@W1 + (L Tx1)@(2 W2) + b.
All device compute in bf16 with fp32 PSUM accumulation.
"""

import sys
import types

sys.path.insert(0, "/opt/trn_rl_repo")

import ml_dtypes
import numpy as np

# antenv.axon_hooks shim (lets run_bass_kernel_spmd(trace=True) profile)
try:
    import trn_agent_boot.trn_boot as _tb

    if "antenv.axon_hooks" not in sys.modules:
        _hook = _tb._ntff_profile_via_ctypes("/opt/axon/libaxon_pjrt.so")
        _m = types.ModuleType("antenv.axon_hooks")
        _m.get_axon_ntff_profile_hook = lambda: _hook
        _m.set_axon_ntff_profile_hook = lambda h: None
        sys.modules["antenv.axon_hooks"] = _m
except Exception:
    pass

import concourse.bass as bass
import concourse.mybir as mybir
import concourse.tile as tile
from concourse import bacc
from concourse.bass_utils import run_bass_kernel_spmd
from concourse.masks import make_identity

BF16 = ml_dtypes.bfloat16
FP8 = ml_dtypes.float8_e4m3  # matches mybir.dt.float8e4
NCORES = 8
G = 256
GPC_OUT = G // NCORES  # graphs per core = 32
H = 200
FIN = 64
POOL_GPC = 8  # graphs per pooling gather call
P = 128
TG = 5  # dst tiles per merged gather call

_cache = {}


def _wrap_idx(vals):
    """[n] int -> [128, n//16] int16 in dma_gather wrapped+replicated layout."""
    n = len(vals)
    assert n % 16 == 0
    w16 = np.asarray(vals, np.int16).reshape(n // 16, 16).T  # [16, n/16]
    return np.tile(w16, (8, 1))  # [128, n/16]


def _preprocess(x, edge_index, batch, lmax):
    N = x.shape[0]
    E = edge_index.shape[1]
    src = edge_index[0].astype(np.int64)
    dst = edge_index[1].astype(np.int64)
    batch = batch.astype(np.int64)

    # --- edge weights (mirror reference, fp32) ---
    deg = np.bincount(src, minlength=N).astype(np.float32)
    dis = np.where(deg > 0, np.maximum(deg, 1.0) ** -0.5, 0.0).astype(np.float32)
    scale = (2.0 / lmax).astype(np.float32)  # [G]
    w_edge = (-dis[src] * dis[dst] * scale[batch[src]]).astype(np.float32)
    diag = (scale[batch] - 1.0).astype(np.float32)  # [N]

    # --- node partition: core c owns graphs [32c, 32c+32) ---
    node_core = (batch // GPC_OUT).astype(np.int64)
    counts = np.bincount(node_core, minlength=NCORES)
    assert counts.min() > 0
    NL = int(np.ceil((counts.max() + 1) / P) * P)
    NT = NL // P
    core_start = np.zeros(NCORES + 1, np.int64)
    core_start[1:] = np.cumsum(counts)
    slot = np.arange(N) - core_start[node_core]  # local slot (natural order)
    g_row = node_core * NL + slot  # row in gathered tensors

    # --- full edge list incl self edges (i, i, diag_i) ---
    asrc = np.concatenate([src, np.arange(N)])
    adst = np.concatenate([dst, np.arange(N)])
    aw = np.concatenate([w_edge, diag]).astype(np.float32)

    e_core = node_core[adst]  # owning core (by dst)
    e_tile = (slot[adst] >> 7).astype(np.int64)
    e_dl = (slot[adst] & 127).astype(np.int64)
    e_half = (node_core[asrc] >= 4).astype(np.int64)  # 0: rows<4NL, 1: rest
    e_grow = g_row[asrc]

    # group by (core, tile, half); order within group arbitrary
    order = np.lexsort((e_half, e_tile, e_core))
    gkey = ((e_core * NT + e_tile) * 2 + e_half)[order]
    # within-group rank
    grp_start_mask = np.ones(len(gkey), bool)
    grp_start_mask[1:] = gkey[1:] != gkey[:-1]
    grp_idx = np.flatnonzero(grp_start_mask)
    within = np.arange(len(gkey)) - np.repeat(grp_idx, np.diff(np.append(grp_idx, len(gkey))))
    cnts = np.zeros(NCORES * NT * 2, np.int64)
    uk, uc = np.unique(gkey, return_counts=True)
    cnts[uk] = uc
    cntA = cnts.reshape(-1, 2)[:, 0]
    cntB = cnts.reshape(-1, 2)[:, 1]
    cA = int(np.ceil(cntA.max() / P))
    cB = int(np.ceil(cntB.max() / P))
    C = cA + cB

    # slot position of each edge inside the per-core [NT, C*128] array
    pos_in_tile = np.where(gkey % 2 == 0, within, cA * P + within)
    pos = (gkey // 2 % NT) * (C * P) + pos_in_tile
    ecore_sorted = e_core[order]

    idx_arr = np.zeros((NCORES, NT * C * P), np.int64)  # gathered-row index
    dl_arr = np.zeros((NCORES, NT * C * P), np.float32)
    w_arr = np.zeros((NCORES, NT * C * P), np.float32)
    grow_adj = np.where(e_half == 1, e_grow - 4 * NL, e_grow)
    idx_arr[ecore_sorted, pos] = grow_adj[order]
    dl_arr[ecore_sorted, pos] = e_dl[order]
    w_arr[ecore_sorted, pos] = aw[order]
    assert idx_arr.max() < 32768

    # device layouts: indices grouped per supertile of TG dst tiles
    # (one lo-half + one hi-half gather call per supertile)
    assert NT % TG == 0
    NS = NT // TG
    idx_dev, mm_dev = [], []
    for c in range(NCORES):
        a3 = idx_arr[c].reshape(NT, C, P)
        cols = []
        for s in range(NS):
            cols.append(_wrap_idx(a3[s * TG : (s + 1) * TG, :cA].reshape(-1)))
            cols.append(_wrap_idx(a3[s * TG : (s + 1) * TG, cA:].reshape(-1)))
        idx_dev.append(np.hstack(cols))  # [128, NT*C*8]
        # expanded selection matrices: mm[t, e, k*128+d] = w * (dstloc==d)
        mm = np.zeros((NT, P, C * P), FP8)
        kk = np.arange(NT * C * P)
        t_of = kk // (C * P)
        k_of = (kk % (C * P)) // P
        e_of = kk % P
        mm[t_of, e_of, k_of * P + dl_arr[c].astype(np.int64)] = w_arr[c].astype(FP8)
        mm_dev.append(mm)

    # --- x layouts ---
    xg = np.zeros((NCORES * NL, P), BF16)
    xg[g_row, :FIN] = x.astype(BF16)
    xloc = [np.ascontiguousarray(xg[c * NL : (c + 1) * NL]) for c in range(NCORES)]

    # --- pooling windows ---
    gcnt = np.bincount(batch, minlength=G).astype(np.int64)
    assert gcnt.min() > 0
    GW = int(np.ceil(gcnt.max() / 16) * 16)
    n_calls = GPC_OUT // POOL_GPC  # 4
    pidx = []
    cntr = []
    for c in range(NCORES):
        zrow = int(counts[c])  # first pad slot, rows are zero
        mean_cols, max_cols = [], []
        for call in range(n_calls):
            mvals = np.zeros(POOL_GPC * GW, np.int64)
            xvals = np.zeros(POOL_GPC * GW, np.int64)
            for gg in range(POOL_GPC):
                g_id = c * GPC_OUT + call * POOL_GPC + gg
                # nodes of graph g_id are a contiguous range (batch sorted)
                lo = np.searchsorted(batch, g_id, "left")
                hi = np.searchsorted(batch, g_id, "right")
                rows = slot[lo:hi]
                k = hi - lo
                mvals[gg * GW : gg * GW + k] = rows
                mvals[gg * GW + k : (gg + 1) * GW] = zrow
                xvals[gg * GW : gg * GW + k] = rows
                xvals[gg * GW + k : (gg + 1) * GW] = rows[0]
            mean_cols.append(_wrap_idx(mvals))
            max_cols.append(_wrap_idx(xvals))
        pidx.append(np.hstack(mean_cols + max_cols))  # [128, 8*GPC*GW/16]
        # replicated across all 128 partitions (free-dim per-graph scale)
        cr = (1.0 / np.maximum(gcnt[c * GPC_OUT : (c + 1) * GPC_OUT], 1.0)).astype(
            np.float32
        )
        cntr.append(np.tile(cr.reshape(1, GPC_OUT), (P, 1)))

    return dict(
        NL=NL, NT=NT, C=C, cA=cA, cB=cB, GW=GW,
        idx=idx_dev, mm=mm_dev, xg=xg, xloc=xloc,
        pidx=pidx, cntr=cntr,
    )


def _pack_weights(W1, W2, W3, W4, b1, b2, b3, b4, fc_w, fc_b):
    def cheb_pack(W, kin_chunks):
        # W [3, Fin, 200] -> W' terms [(W0-W2), W1, 2*W2]; pad to [3, kc, 128, 256]
        Wp = np.stack([W[0] - W[2], W[1], 2.0 * W[2]]).astype(np.float32)
        out = np.zeros((3, kin_chunks, P, 256), np.float32)
        fin = W.shape[1]
        for ki in range(kin_chunks):
            lo = ki * P
            hi = min(fin, lo + P)
            if hi > lo:
                out[:, ki, : hi - lo, :H] = Wp[:, lo:hi, :]
        return out.astype(BF16)

    w1 = cheb_pack(W1, 1)
    w2 = cheb_pack(W2, 2)
    w3 = cheb_pack(W3, 2)
    w4 = cheb_pack(W4, 2)
    bvec = np.zeros((P, 4, 2), np.float32)
    for li, b in enumerate([b1, b2, b3, b4]):
        for fo in range(2):
            seg = b[fo * P : min(H, (fo + 1) * P)]
            bvec[: len(seg), li, fo] = seg
    fcw = np.zeros((P, 4, 2), np.float32)
    fcw[:, 0] = fc_w[0:P]
    fcw[: H - P, 1] = fc_w[P:H]
    fcw[:, 2] = fc_w[H : H + P]
    fcw[: H - P, 3] = fc_w[H + P : 2 * H]
    fcb = np.tile(fc_b.astype(np.float32).reshape(1, 2), (GPC_OUT, 1))
    return dict(
        w1=w1, w2=w2, w3=w3, w4=w4, bvec=bvec, fcw=fcw.astype(BF16), fcb=fcb
    )


def _build(NL, NT, C, cA, cB, GW, stage=None):
    """Build the SPMD kernel graph (identical for all cores)."""
    F32, BF, I16 = mybir.dt.float32, mybir.dt.bfloat16, mybir.dt.int16
    F8 = mybir.dt.float8e4
    nc = bacc.Bacc(None, num_devices=NCORES, num_swdge_queues=4)
    rg = [list(range(NCORES))]
    n_calls = GPC_OUT // POOL_GPC
    NS = NT // TG

    # inputs
    d_xg = nc.declare_dram_parameter("xg", [NCORES * NL, P], BF, isOutput=False)
    d_xloc = nc.declare_dram_parameter("xloc", [NL, P], BF, isOutput=False)
    d_idx = nc.declare_dram_parameter("idx", [P, NT * C * 8], I16, isOutput=False)
    d_mm = nc.declare_dram_parameter("mm", [NT, P, C * P], F8, isOutput=False)
    d_pidx = nc.declare_dram_parameter(
        "pidx", [P, 2 * n_calls * POOL_GPC * GW // 16], I16, isOutput=False
    )
    d_cntr = nc.declare_dram_parameter("cntr", [P, GPC_OUT], F32, isOutput=False)
    d_w1 = nc.declare_dram_parameter("w1", [3, 1, P, 256], BF, isOutput=False)
    d_w2 = nc.declare_dram_parameter("w2", [3, 2, P, 256], BF, isOutput=False)
    d_w3 = nc.declare_dram_parameter("w3", [3, 2, P, 256], BF, isOutput=False)
    d_w4 = nc.declare_dram_parameter("w4", [3, 2, P, 256], BF, isOutput=False)
    d_bvec = nc.declare_dram_parameter("bvec", [P, 4, 2], F32, isOutput=False)
    d_fcw = nc.declare_dram_parameter("fcw", [P, 4, 2], BF, isOutput=False)
    d_fcb = nc.declare_dram_parameter("fcb", [GPC_OUT, 2], F32, isOutput=False)
    d_out = nc.declare_dram_parameter("out", [GPC_OUT, 2], F32, isOutput=True)

    # internal DRAM (gather/AG payloads in fp8; pooling payload bf16)
    bounce = nc.dram_tensor("bounce", [NL, 256], F8)
    bounce16 = nc.dram_tensor("bounce16", [NL, 256], BF)
    bounce_s = nc.dram_tensor("bounce_s", [NL, P], BF)
    hg = nc.dram_tensor("hg", [NCORES * NL, 256], F8, addr_space="Shared")
    t1g = nc.dram_tensor("t1g", [NCORES * NL, 256], F8, addr_space="Shared")
    t1g_s = nc.dram_tensor("t1g_s", [NCORES * NL, P], BF, addr_space="Shared")

    with tile.TileContext(nc) as tc:
        with (
            tc.tile_pool(name="const", bufs=1) as cp,
            tc.tile_pool(name="big", bufs=1) as bigp,
            tc.tile_pool(name="work", bufs=3) as wp,
            tc.tile_pool(name="ypool", bufs=2) as yp,
            tc.tile_pool(name="ygath", bufs=2) as ygp,
            tc.tile_pool(name="mpool", bufs=3) as mp,
            tc.tile_pool(name="psA", bufs=3, space="PSUM") as psA,
            tc.tile_pool(name="psB", bufs=2, space="PSUM") as psB,
            tc.tile_pool(name="psC", bufs=2, space="PSUM") as psC,
            tc.tile_pool(name="psD", bufs=1, space="PSUM") as psD,
        ):
            # ---- resident constants ----
            idx_sb = cp.tile([P, NT * C * 8], I16)
            nc.sync.dma_start(out=idx_sb[:], in_=d_idx.ap())
            pidx_sb = cp.tile([P, 2 * n_calls * POOL_GPC * GW // 16], I16)
            nc.sync.dma_start(out=pidx_sb[:], in_=d_pidx.ap())
            w_sb = {}
            for nm, dp, kc in (("w1", d_w1, 1), ("w2", d_w2, 2), ("w3", d_w3, 2), ("w4", d_w4, 2)):
                t = cp.tile([P, 3, kc, 256], BF, tag=nm)
                nc.sync.dma_start(
                    out=t[:], in_=dp.ap().rearrange("t k p f -> p t k f")
                )
                w_sb[nm] = t
            bvec_sb = cp.tile([P, 4, 2], F32)
            nc.sync.dma_start(out=bvec_sb[:], in_=d_bvec.ap())
            fcw_sb = cp.tile([P, 4, 2], BF)
            nc.sync.dma_start(out=fcw_sb[:], in_=d_fcw.ap())
            fcb_sb = cp.tile([GPC_OUT, 2], F32)
            nc.sync.dma_start(out=fcb_sb[:], in_=d_fcb.ap())
            cntr_sb = cp.tile([P, GPC_OUT], F32)
            nc.sync.dma_start(out=cntr_sb[:], in_=d_cntr.ap())

            ident = cp.tile([P, P], BF)
            make_identity(nc, ident[:])

            # ---- feature-major locals ----
            hT = [bigp.tile([P, 2, NT * P], BF, tag=f"hT{i}", name=f"hT{i}") for i in range(2)]
            t1T = bigp.tile([P, 2, NT * P], BF, tag="t1T")
            p2T = bigp.tile([P, 2, NT * P], BF, tag="p2T")
            for buf in (hT[0], hT[1], t1T, p2T):
                nc.vector.memset(buf[:], 0.0)
            zt = cp.tile([P, 256], BF)
            nc.vector.memset(zt[:], 0.0)
            zt8 = cp.tile([P, 256], F8)
            nc.vector.memset(zt8[:], 0.0)
            for t in range(NT):
                nc.sync.dma_start(
                    out=bounce.ap()[t * P : (t + 1) * P, :], in_=zt8[:]
                )
                nc.sync.dma_start(
                    out=bounce16.ap()[t * P : (t + 1) * P, :], in_=zt[:]
                )

            # x -> xT (= hT[0] chunk 0)
            for t in range(NT):
                xt = wp.tile([P, P], BF, tag="xload")
                nc.sync.dma_start(out=xt[:], in_=d_xloc.ap()[t * P : (t + 1) * P, :])
                pt = psB.tile([P, P], BF, tag="tp")
                nc.tensor.transpose(pt[:], xt[:], ident[:])
                nc.vector.tensor_copy(out=hT[0][:, 0, t * P : (t + 1) * P], in_=pt[:])

            def prop(src_dram, EW, NW, outT, bounce_dram, ydt):
                """outT[f, n] = sum_e w[e] h_src[e, f] per dst tile; optionally
                also write node-major rows to bounce_dram (fp8). Gathers are
                merged: one lo + one hi call per supertile of TG dst tiles."""
                lo = src_dram.ap()[0 : 4 * NL, :]
                hi = src_dram.ap()[4 * NL : 8 * NL, :]
                for s in range(NS):
                    y = ygp.tile([P, TG * C, EW], ydt, tag="Y")
                    base = s * TG * C * 8
                    nc.gpsimd.dma_gather(
                        out_ap=y[:, 0 : TG * cA, :],
                        in_ap=lo,
                        idxs_ap=idx_sb[:, base : base + TG * cA * 8],
                        num_idxs=TG * cA * P,
                        num_idxs_reg=TG * cA * P,
                        elem_size=EW,
                        single_packet=False,
                        queue_num=s % 4,
                    )
                    nc.gpsimd.dma_gather(
                        out_ap=y[:, TG * cA : TG * C, :],
                        in_ap=hi,
                        idxs_ap=idx_sb[:, base + TG * cA * 8 : base + TG * C * 8],
                        num_idxs=TG * cB * P,
                        num_idxs_reg=TG * cB * P,
                        elem_size=EW,
                        single_packet=False,
                        queue_num=(s + 2) % 4,
                    )
                    for tt in range(TG):
                        t = s * TG + tt
                        mt = mp.tile([P, C, P], F8, tag="mt")
                        nc.sync.dma_start(
                            out=mt[:],
                            in_=d_mm.ap()[t].rearrange("e (k d) -> e k d", d=P),
                        )
                        acc = psA.tile([P, NW], F32, tag="acc")
                        for k in range(C):
                            ysl = (
                                y[:, tt * cA + k, 0:NW]
                                if k < cA
                                else y[:, TG * cA + tt * cB + (k - cA), 0:NW]
                            )
                            nc.tensor.matmul(
                                acc[:],
                                lhsT=mt[:, k, :],
                                rhs=ysl,
                                start=(k == 0),
                                stop=(k == C - 1),
                            )
                        nm = wp.tile([P, NW], BF, tag="nm")
                        nc.vector.tensor_copy(out=nm[:], in_=acc[:])
                        nfc = (NW + P - 1) // P
                        for fc in range(nfc):
                            w_fc = min(P, NW - fc * P)
                            pt = psB.tile([P, P], BF, tag="tp")
                            nc.tensor.transpose(
                                pt[:w_fc, :],
                                nm[:, fc * P : fc * P + w_fc],
                                ident[:],
                            )
                            nc.vector.tensor_copy(
                                out=outT[:w_fc, fc, t * P : (t + 1) * P],
                                in_=pt[:w_fc, :],
                            )
                        if bounce_dram is not None:
                            if bounce_dram is bounce_s:
                                nc.sync.dma_start(
                                    out=bounce_dram.ap()[t * P : (t + 1) * P, 0:NW],
                                    in_=nm[:],
                                )
                            else:
                                nm8 = wp.tile([P, NW], F8, tag="nm8")
                                nc.vector.tensor_copy(out=nm8[:], in_=acc[:])
                                nc.sync.dma_start(
                                    out=bounce_dram.ap()[t * P : (t + 1) * P, 0:NW],
                                    in_=nm8[:],
                                )

            def dense(l_idx, wt, kc, inT0, h_out, last=False):
                """h_out = relu(Tx0@W'0 + Tx1@W'1 + P2@W'2 + b), feature-major;
                also write node-major tiles to bounce (fp8) or bounce16."""
                terms = [(inT0, 0), (t1T, 1), (p2T, 2)]
                for t in range(NT):
                    nm = wp.tile([P, 256], BF if last else F8, tag="nmd")
                    for fo in range(2):
                        pd = psC.tile([P, P], F32, tag="pd")
                        n_mm = len(terms) * kc
                        i_mm = 0
                        for inT, term in terms:
                            for ki in range(kc):
                                nc.tensor.matmul(
                                    pd[:],
                                    lhsT=w_sb[wt][:, term, ki, fo * P : (fo + 1) * P],
                                    rhs=inT[:, ki, t * P : (t + 1) * P],
                                    start=(i_mm == 0),
                                    stop=(i_mm == n_mm - 1),
                                )
                                i_mm += 1
                        nc.scalar.activation(
                            h_out[:, fo, t * P : (t + 1) * P],
                            pd[:],
                            mybir.ActivationFunctionType.Relu,
                            bias=bvec_sb[:, l_idx, fo : fo + 1],
                        )
                        pt = psB.tile([P, P], BF, tag="tp")
                        nc.tensor.transpose(
                            pt[:], h_out[:, fo, t * P : (t + 1) * P], ident[:]
                        )
                        nc.vector.tensor_copy(
                            out=nm[:, fo * P : (fo + 1) * P], in_=pt[:]
                        )
                    nc.sync.dma_start(
                        out=(bounce16 if last else bounce).ap()[
                            t * P : (t + 1) * P, :
                        ],
                        in_=nm[:],
                    )

            def allgather(src, dst):
                nc.gpsimd.collective_compute(
                    "AllGather",
                    mybir.AluOpType.bypass,
                    replica_groups=rg,
                    ins=[src.ap().opt()],
                    outs=[dst.ap().opt()],
                )

            # ================= layer 1 (input x, width 64->200) =================
            prop(d_xg, P, FIN, t1T, bounce_s, BF)
            allgather(bounce_s, t1g_s)
            prop(t1g_s, P, FIN, p2T, None, BF)
            dense(0, "w1", 1, hT[0], hT[1])
            allgather(bounce, hg)

            # ================= layers 2..4 =================
            cur = 1
            for li, wt in ((1, "w2"), (2, "w3"), (3, "w4")):
                prop(hg, 256, H, t1T, bounce, F8)
                allgather(bounce, t1g)
                prop(t1g, 256, H, p2T, None, F8)
                dense(li, wt, 2, hT[cur], hT[1 - cur], last=(li == 3))
                cur = 1 - cur
                if li < 3:
                    allgather(bounce, hg)

            if True:
                # ================= pooling + fc + log_softmax =================
                red = {}
                for typ in range(2):  # 0: mean(sum), 1: max
                    rT = bigp.tile([P, 2, GPC_OUT], F32, tag=f"red{typ}")
                    red[typ] = rT
                    for call in range(n_calls):
                        py = yp.tile([P, 2, POOL_GPC * GW], BF, tag="poolY")
                        base = (typ * n_calls + call) * (POOL_GPC * GW // 16)
                        nc.gpsimd.dma_gather(
                            out_ap=py[:],
                            in_ap=bounce16.ap(),
                            idxs_ap=pidx_sb[:, base : base + POOL_GPC * GW // 16],
                            num_idxs=POOL_GPC * GW,
                            num_idxs_reg=POOL_GPC * GW,
                            elem_size=256,
                            transpose=True,
                            single_packet=False,
                        )
                        for gg in range(POOL_GPC):
                            gcol = call * POOL_GPC + gg
                            for fc in range(2):
                                nc.vector.tensor_reduce(
                                    out=rT[:, fc, gcol : gcol + 1],
                                    in_=py[:, fc, gg * GW : (gg + 1) * GW],
                                    axis=mybir.AxisListType.X,
                                    op=mybir.AluOpType.add if typ == 0 else mybir.AluOpType.max,
                                )
                # mean scale + cast to bf16 lhsT tiles
                pool_bf = bigp.tile([P, 4, GPC_OUT], BF, tag="poolbf")
                for fc in range(2):
                    nc.vector.tensor_tensor(
                        out=pool_bf[:, fc, :],
                        in0=red[0][:, fc, :],
                        in1=cntr_sb[:, :],
                        op=mybir.AluOpType.mult,
                    )
                    nc.vector.tensor_copy(out=pool_bf[:, 2 + fc, :], in_=red[1][:, fc, :])
                pfc = psD.tile([GPC_OUT, 2], F32, tag="pfc")
                for j in range(4):
                    nc.tensor.matmul(
                        pfc[:],
                        lhsT=pool_bf[:, j, :],
                        rhs=fcw_sb[:, j, :],
                        start=(j == 0),
                        stop=(j == 3),
                    )
                z = wp.tile([GPC_OUT, 2], F32, tag="z")
                nc.vector.tensor_tensor(
                    out=z[:], in0=pfc[:], in1=fcb_sb[:, :],
                    op=mybir.AluOpType.add,
                )
                mx = wp.tile([GPC_OUT, 1], F32, tag="mx")
                nc.vector.tensor_reduce(
                    out=mx[:], in_=z[:], axis=mybir.AxisListType.X, op=mybir.AluOpType.max
                )
                zm = wp.tile([GPC_OUT, 2], F32, tag="zm")
                nc.vector.tensor_scalar(
                    zm[:], z[:], mx[:], None, mybir.AluOpType.subtract
                )
                ez = wp.tile([GPC_OUT, 2], F32, tag="ez")
                nc.scalar.activation(ez[:], zm[:], mybir.ActivationFunctionType.Exp)
                sz = wp.tile([GPC_OUT, 1], F32, tag="sz")
                nc.vector.tensor_reduce(
                    out=sz[:], in_=ez[:], axis=mybir.AxisListType.X, op=mybir.AluOpType.add
                )
                lz = wp.tile([GPC_OUT, 1], F32, tag="lz")
                nc.scalar.activation(lz[:], sz[:], mybir.ActivationFunctionType.Ln)
                oz = wp.tile([GPC_OUT, 2], F32, tag="oz")
                nc.vector.tensor_scalar(
                    oz[:], zm[:], lz[:], None, mybir.AluOpType.subtract
                )
                nc.sync.dma_start(out=d_out.ap(), in_=oz[:])

    nc.finalize()
    return nc


def kernel(**inputs):
    x = np.asarray(inputs["x"], np.float32)
    edge_index = np.asarray(inputs["edge_index"])
    batch = np.asarray(inputs["batch"])
    lmax = np.asarray(inputs["lmax"], np.float32)

    pp = _preprocess(x, edge_index, batch, lmax)
    wts = _pack_weights(
        np.asarray(inputs["W1"], np.float32), np.asarray(inputs["W2"], np.float32),
        np.asarray(inputs["W3"], np.float32), np.asarray(inputs["W4"], np.float32),
        np.asarray(inputs["b1"], np.float32), np.asarray(inputs["b2"], np.float32),
        np.asarray(inputs["b3"], np.float32), np.asarray(inputs["b4"], np.float32),
        np.asarray(inputs["fc_w"], np.float32), np.asarray(inputs["fc_b"], np.float32),
    )

    key = (pp["NL"], pp["NT"], pp["C"], pp["cA"], pp["cB"], pp["GW"])
    if key not in _cache:
        _cache[key] = _build(*key)
    nc = _cache[key]

    shared = dict(
        xg=pp["xg"],
        w1=wts["w1"], w2=wts["w2"], w3=wts["w3"], w4=wts["w4"],
        bvec=wts["bvec"], fcw=wts["fcw"], fcb=wts["fcb"],
    )
    in_maps = [
        dict(
            shared,
            xloc=pp["xloc"][c], idx=pp["idx"][c], mm=pp["mm"][c],
            pidx=pp["pidx"][c], cntr=pp["cntr"][c],
        )
        for c in range(NCORES)
    ]
    trace = bool(int(__import__("os").environ.get("KERNEL_TRACE", "0")))
    res = run_bass_kernel_spmd(nc, in_maps, list(range(NCORES)), trace=trace)
    if trace:
        kernel.last_exec_time_ns = res.exec_time_ns
        kernel.last_results = res
    out = np.concatenate([res.results[c]["out"] for c in range(NCORES)], axis=0)
    return out.astype(np.float32)


kernel.last_exec_time_ns = None



# revision 13
# speedup vs baseline: 1.2321x; 1.2321x over previous
"""ChebNet (K=3, 4 layers, H=200) on 8 TRN2 NeuronCores.

Strategy: data-parallel over graphs (32 graphs / core, contiguous node
ranges since batch is sorted). The sparse propagation L_hat@h is done as
  gather h[src] rows (dma_gather from a replicated copy in HBM)
  -> segmented matmul against an on-chip-built selection matrix M
     (M[e, d] = w[e] * (dst_local[e] == d)) accumulating per dst tile.
Replication of h across cores between propagations via AllGather.
The Chebyshev recurrence is refactored to avoid the *2/-Tx0 fixups:
  out = Tx0@(W0-W2) + Tx1@W1 + (L Tx1)@(2 W2) + b.
All device compute in bf16 with fp32 PSUM accumulation.
"""

import sys
import types

sys.path.insert(0, "/opt/trn_rl_repo")

import ml_dtypes
import numpy as np

# antenv.axon_hooks shim (lets run_bass_kernel_spmd(trace=True) profile)
try:
    import trn_agent_boot.trn_boot as _tb

    if "antenv.axon_hooks" not in sys.modules:
        _hook = _tb._ntff_profile_via_ctypes("/opt/axon/libaxon_pjrt.so")
        _m = types.ModuleType("antenv.axon_hooks")
        _m.get_axon_ntff_profile_hook = lambda: _hook
        _m.set_axon_ntff_profile_hook = lambda h: None
        sys.modules["antenv.axon_hooks"] = _m
except Exception:
    pass

import concourse.bass as bass
import concourse.mybir as mybir
import concourse.tile as tile
from concourse import bacc
from concourse.bass_utils import run_bass_kernel_spmd
from concourse.masks import make_identity

BF16 = ml_dtypes.bfloat16
FP8 = ml_dtypes.float8_e4m3  # matches mybir.dt.float8e4
NCORES = 8
G = 256
GPC_OUT = G // NCORES  # graphs per core = 32
H = 200
FIN = 64
POOL_GPC = 8  # graphs per pooling gather call
P = 128
TG = 2  # dst tiles per merged gather call

_cache = {}


def _wrap_idx(vals):
    """[n] int -> [128, n//16] int16 in dma_gather wrapped+replicated layout."""
    n = len(vals)
    assert n % 16 == 0
    w16 = np.asarray(vals, np.int16).reshape(n // 16, 16).T  # [16, n/16]
    return np.tile(w16, (8, 1))  # [128, n/16]


def _preprocess(x, edge_index, batch, lmax):
    N = x.shape[0]
    E = edge_index.shape[1]
    src = edge_index[0].astype(np.int64)
    dst = edge_index[1].astype(np.int64)
    batch = batch.astype(np.int64)

    # --- edge weights (mirror reference, fp32) ---
    deg = np.bincount(src, minlength=N).astype(np.float32)
    dis = np.where(deg > 0, np.maximum(deg, 1.0) ** -0.5, 0.0).astype(np.float32)
    scale = (2.0 / lmax).astype(np.float32)  # [G]
    w_edge = (-dis[src] * dis[dst] * scale[batch[src]]).astype(np.float32)
    diag = (scale[batch] - 1.0).astype(np.float32)  # [N]

    # --- node partition: core c owns graphs [32c, 32c+32) ---
    node_core = (batch // GPC_OUT).astype(np.int64)
    counts = np.bincount(node_core, minlength=NCORES)
    assert counts.min() > 0
    NL = int(np.ceil((counts.max() + 1) / P) * P)
    NT = NL // P
    core_start = np.zeros(NCORES + 1, np.int64)
    core_start[1:] = np.cumsum(counts)
    slot = np.arange(N) - core_start[node_core]  # local slot (natural order)
    g_row = node_core * NL + slot  # row in gathered tensors

    # --- full edge list incl self edges (i, i, diag_i) ---
    asrc = np.concatenate([src, np.arange(N)])
    adst = np.concatenate([dst, np.arange(N)])
    aw = np.concatenate([w_edge, diag]).astype(np.float32)

    e_core = node_core[adst]  # owning core (by dst)
    e_tile = (slot[adst] >> 7).astype(np.int64)
    e_dl = (slot[adst] & 127).astype(np.int64)
    e_half = (node_core[asrc] >= 4).astype(np.int64)  # 0: rows<4NL, 1: rest
    e_grow = g_row[asrc]

    # group by (core, tile, half); order within group arbitrary
    order = np.lexsort((e_half, e_tile, e_core))
    gkey = ((e_core * NT + e_tile) * 2 + e_half)[order]
    # within-group rank
    grp_start_mask = np.ones(len(gkey), bool)
    grp_start_mask[1:] = gkey[1:] != gkey[:-1]
    grp_idx = np.flatnonzero(grp_start_mask)
    within = np.arange(len(gkey)) - np.repeat(grp_idx, np.diff(np.append(grp_idx, len(gkey))))
    cnts = np.zeros(NCORES * NT * 2, np.int64)
    uk, uc = np.unique(gkey, return_counts=True)
    cnts[uk] = uc
    cntA = cnts.reshape(-1, 2)[:, 0]
    cntB = cnts.reshape(-1, 2)[:, 1]
    cA = int(np.ceil(cntA.max() / P))
    cB = int(np.ceil(cntB.max() / P))
    C = cA + cB

    # slot position of each edge inside the per-core [NT, C*128] array
    pos_in_tile = np.where(gkey % 2 == 0, within, cA * P + within)
    pos = (gkey // 2 % NT) * (C * P) + pos_in_tile
    ecore_sorted = e_core[order]

    idx_arr = np.zeros((NCORES, NT * C * P), np.int64)  # gathered-row index
    dl_arr = np.zeros((NCORES, NT * C * P), np.float32)
    w_arr = np.zeros((NCORES, NT * C * P), np.float32)
    grow_adj = np.where(e_half == 1, e_grow - 4 * NL, e_grow)
    idx_arr[ecore_sorted, pos] = grow_adj[order]
    dl_arr[ecore_sorted, pos] = e_dl[order]
    w_arr[ecore_sorted, pos] = aw[order]
    assert idx_arr.max() < 32768

    # device layouts: indices grouped per supertile of TG dst tiles
    # (one lo-half + one hi-half gather call per supertile)
    assert NT % TG == 0
    NS = NT // TG
    idx_dev, mm_dev = [], []
    for c in range(NCORES):
        a3 = idx_arr[c].reshape(NT, C, P)
        cols = []
        for s in range(NS):
            cols.append(_wrap_idx(a3[s * TG : (s + 1) * TG, :cA].reshape(-1)))
            cols.append(_wrap_idx(a3[s * TG : (s + 1) * TG, cA:].reshape(-1)))
        idx_dev.append(np.hstack(cols))  # [128, NT*C*8]
        # expanded selection matrices: mm[t, e, k*128+d] = w * (dstloc==d)
        mm = np.zeros((NT, P, C * P), FP8)
        kk = np.arange(NT * C * P)
        t_of = kk // (C * P)
        k_of = (kk % (C * P)) // P
        e_of = kk % P
        mm[t_of, e_of, k_of * P + dl_arr[c].astype(np.int64)] = w_arr[c].astype(FP8)
        mm_dev.append(mm)

    # --- x layouts ---
    xg = np.zeros((NCORES * NL, P), BF16)
    xg[g_row, :FIN] = x.astype(BF16)
    xloc = [np.ascontiguousarray(xg[c * NL : (c + 1) * NL]) for c in range(NCORES)]

    # --- pooling windows ---
    gcnt = np.bincount(batch, minlength=G).astype(np.int64)
    assert gcnt.min() > 0
    GW = int(np.ceil(gcnt.max() / 16) * 16)
    n_calls = GPC_OUT // POOL_GPC  # 4
    pidx = []
    cntr = []
    for c in range(NCORES):
        zrow = int(counts[c])  # first pad slot, rows are zero
        mean_cols, max_cols = [], []
        for call in range(n_calls):
            mvals = np.zeros(POOL_GPC * GW, np.int64)
            xvals = np.zeros(POOL_GPC * GW, np.int64)
            for gg in range(POOL_GPC):
                g_id = c * GPC_OUT + call * POOL_GPC + gg
                # nodes of graph g_id are a contiguous range (batch sorted)
                lo = np.searchsorted(batch, g_id, "left")
                hi = np.searchsorted(batch, g_id, "right")
                rows = slot[lo:hi]
                k = hi - lo
                mvals[gg * GW : gg * GW + k] = rows
                mvals[gg * GW + k : (gg + 1) * GW] = zrow
                xvals[gg * GW : gg * GW + k] = rows
                xvals[gg * GW + k : (gg + 1) * GW] = rows[0]
            mean_cols.append(_wrap_idx(mvals))
            max_cols.append(_wrap_idx(xvals))
        pidx.append(np.hstack(mean_cols + max_cols))  # [128, 8*GPC*GW/16]
        # replicated across all 128 partitions (free-dim per-graph scale)
        cr = (1.0 / np.maximum(gcnt[c * GPC_OUT : (c + 1) * GPC_OUT], 1.0)).astype(
            np.float32
        )
        cntr.append(np.tile(cr.reshape(1, GPC_OUT), (P, 1)))

    return dict(
        NL=NL, NT=NT, C=C, cA=cA, cB=cB, GW=GW,
        idx=idx_dev, mm=mm_dev, xg=xg, xloc=xloc,
        pidx=pidx, cntr=cntr,
    )


def _pack_weights(W1, W2, W3, W4, b1, b2, b3, b4, fc_w, fc_b):
    def cheb_pack(W, kin_chunks):
        # W [3, Fin, 200] -> W' terms [(W0-W2), W1, 2*W2]; pad to [3, kc, 128, 256]
        Wp = np.stack([W[0] - W[2], W[1], 2.0 * W[2]]).astype(np.float32)
        out = np.zeros((3, kin_chunks, P, 256), np.float32)
        fin = W.shape[1]
        for ki in range(kin_chunks):
            lo = ki * P
            hi = min(fin, lo + P)
            if hi > lo:
                out[:, ki, : hi - lo, :H] = Wp[:, lo:hi, :]
        return out.astype(BF16)

    w1 = cheb_pack(W1, 1)
    w2 = cheb_pack(W2, 2)
    w3 = cheb_pack(W3, 2)
    w4 = cheb_pack(W4, 2)
    bvec = np.zeros((P, 4, 2), np.float32)
    for li, b in enumerate([b1, b2, b3, b4]):
        for fo in range(2):
            seg = b[fo * P : min(H, (fo + 1) * P)]
            bvec[: len(seg), li, fo] = seg
    fcw = np.zeros((P, 4, 2), np.float32)
    fcw[:, 0] = fc_w[0:P]
    fcw[: H - P, 1] = fc_w[P:H]
    fcw[:, 2] = fc_w[H : H + P]
    fcw[: H - P, 3] = fc_w[H + P : 2 * H]
    fcb = np.tile(fc_b.astype(np.float32).reshape(1, 2), (GPC_OUT, 1))
    return dict(
        w1=w1, w2=w2, w3=w3, w4=w4, bvec=bvec, fcw=fcw.astype(BF16), fcb=fcb
    )


def _build(NL, NT, C, cA, cB, GW, stage=None):
    """Build the SPMD kernel graph (identical for all cores)."""
    F32, BF, I16 = mybir.dt.float32, mybir.dt.bfloat16, mybir.dt.int16
    F8 = mybir.dt.float8e4
    nc = bacc.Bacc(None, num_devices=NCORES, num_swdge_queues=4)
    rg = [list(range(NCORES))]
    n_calls = GPC_OUT // POOL_GPC
    NS = NT // TG

    # inputs
    d_xg = nc.declare_dram_parameter("xg", [NCORES * NL, P], BF, isOutput=False)
    d_xloc = nc.declare_dram_parameter("xloc", [NL, P], BF, isOutput=False)
    d_idx = nc.declare_dram_parameter("idx", [P, NT * C * 8], I16, isOutput=False)
    d_mm = nc.declare_dram_parameter("mm", [NT, P, C * P], F8, isOutput=False)
    d_pidx = nc.declare_dram_parameter(
        "pidx", [P, 2 * n_calls * POOL_GPC * GW // 16], I16, isOutput=False
    )
    d_cntr = nc.declare_dram_parameter("cntr", [P, GPC_OUT], F32, isOutput=False)
    d_w1 = nc.declare_dram_parameter("w1", [3, 1, P, 256], BF, isOutput=False)
    d_w2 = nc.declare_dram_parameter("w2", [3, 2, P, 256], BF, isOutput=False)
    d_w3 = nc.declare_dram_parameter("w3", [3, 2, P, 256], BF, isOutput=False)
    d_w4 = nc.declare_dram_parameter("w4", [3, 2, P, 256], BF, isOutput=False)
    d_bvec = nc.declare_dram_parameter("bvec", [P, 4, 2], F32, isOutput=False)
    d_fcw = nc.declare_dram_parameter("fcw", [P, 4, 2], BF, isOutput=False)
    d_fcb = nc.declare_dram_parameter("fcb", [GPC_OUT, 2], F32, isOutput=False)
    d_out = nc.declare_dram_parameter("out", [GPC_OUT, 2], F32, isOutput=True)

    # internal DRAM (gather/AG payloads in fp8; pooling payload bf16)
    bounce = nc.dram_tensor("bounce", [NL, 256], F8)
    bounce16 = nc.dram_tensor("bounce16", [NL, 256], BF)
    bounce_s = nc.dram_tensor("bounce_s", [NL, P], BF)
    hg = nc.dram_tensor("hg", [NCORES * NL, 256], F8, addr_space="Shared")
    t1g = nc.dram_tensor("t1g", [NCORES * NL, 256], F8, addr_space="Shared")
    t1g_s = nc.dram_tensor("t1g_s", [NCORES * NL, P], BF, addr_space="Shared")

    with tile.TileContext(nc) as tc:
        with (
            tc.tile_pool(name="const", bufs=1) as cp,
            tc.tile_pool(name="big", bufs=1) as bigp,
            tc.tile_pool(name="work", bufs=3) as wp,
            tc.tile_pool(name="ypool", bufs=2) as yp,
            tc.tile_pool(name="ygath", bufs=4) as ygp,
            tc.tile_pool(name="mpool", bufs=3) as mp,
            tc.tile_pool(name="psA", bufs=3, space="PSUM") as psA,
            tc.tile_pool(name="psB", bufs=2, space="PSUM") as psB,
            tc.tile_pool(name="psC", bufs=2, space="PSUM") as psC,
            tc.tile_pool(name="psD", bufs=1, space="PSUM") as psD,
        ):
            # ---- resident constants ----
            idx_sb = cp.tile([P, NT * C * 8], I16)
            nc.sync.dma_start(out=idx_sb[:], in_=d_idx.ap())
            pidx_sb = cp.tile([P, 2 * n_calls * POOL_GPC * GW // 16], I16)
            nc.sync.dma_start(out=pidx_sb[:], in_=d_pidx.ap())
            w_sb = {}
            for nm, dp, kc in (("w1", d_w1, 1), ("w2", d_w2, 2), ("w3", d_w3, 2), ("w4", d_w4, 2)):
                t = cp.tile([P, 3, kc, 256], BF, tag=nm)
                nc.sync.dma_start(
                    out=t[:], in_=dp.ap().rearrange("t k p f -> p t k f")
                )
                w_sb[nm] = t
            bvec_sb = cp.tile([P, 4, 2], F32)
            nc.sync.dma_start(out=bvec_sb[:], in_=d_bvec.ap())
            fcw_sb = cp.tile([P, 4, 2], BF)
            nc.sync.dma_start(out=fcw_sb[:], in_=d_fcw.ap())
            fcb_sb = cp.tile([GPC_OUT, 2], F32)
            nc.sync.dma_start(out=fcb_sb[:], in_=d_fcb.ap())
            cntr_sb = cp.tile([P, GPC_OUT], F32)
            nc.sync.dma_start(out=cntr_sb[:], in_=d_cntr.ap())

            ident = cp.tile([P, P], BF)
            make_identity(nc, ident[:])

            # ---- feature-major locals ----
            hT = [bigp.tile([P, 2, NT * P], BF, tag=f"hT{i}", name=f"hT{i}") for i in range(2)]
            t1T = bigp.tile([P, 2, NT * P], BF, tag="t1T")
            p2T = bigp.tile([P, 2, NT * P], BF, tag="p2T")
            for buf in (hT[0], hT[1], t1T, p2T):
                nc.vector.memset(buf[:], 0.0)
            zt = cp.tile([P, 256], BF)
            nc.vector.memset(zt[:], 0.0)
            zt8 = cp.tile([P, 256], F8)
            nc.vector.memset(zt8[:], 0.0)
            for t in range(NT):
                nc.sync.dma_start(
                    out=bounce.ap()[t * P : (t + 1) * P, :], in_=zt8[:]
                )
                nc.sync.dma_start(
                    out=bounce16.ap()[t * P : (t + 1) * P, :], in_=zt[:]
                )

            # x -> xT (= hT[0] chunk 0)
            for t in range(NT):
                xt = wp.tile([P, P], BF, tag="xload")
                nc.sync.dma_start(out=xt[:], in_=d_xloc.ap()[t * P : (t + 1) * P, :])
                pt = psB.tile([P, P], BF, tag="tp")
                nc.tensor.transpose(pt[:], xt[:], ident[:])
                nc.vector.tensor_copy(out=hT[0][:, 0, t * P : (t + 1) * P], in_=pt[:])

            def prop(src_dram, EW, NW, outT, bounce_dram, ydt):
                """outT[f, n] = sum_e w[e] h_src[e, f] per dst tile; optionally
                also write node-major rows to bounce_dram (fp8). Gathers are
                merged: one lo + one hi call per supertile of TG dst tiles."""
                lo = src_dram.ap()[0 : 4 * NL, :]
                hi = src_dram.ap()[4 * NL : 8 * NL, :]
                for s in range(NS):
                    y = ygp.tile([P, TG * C, EW], ydt, tag="Y")
                    base = s * TG * C * 8
                    nc.gpsimd.dma_gather(
                        out_ap=y[:, 0 : TG * cA, :],
                        in_ap=lo,
                        idxs_ap=idx_sb[:, base : base + TG * cA * 8],
                        num_idxs=TG * cA * P,
                        num_idxs_reg=TG * cA * P,
                        elem_size=EW,
                        single_packet=False,
                        queue_num=s % 4,
                    )
                    nc.gpsimd.dma_gather(
                        out_ap=y[:, TG * cA : TG * C, :],
                        in_ap=hi,
                        idxs_ap=idx_sb[:, base + TG * cA * 8 : base + TG * C * 8],
                        num_idxs=TG * cB * P,
                        num_idxs_reg=TG * cB * P,
                        elem_size=EW,
                        single_packet=False,
                        queue_num=(s + 2) % 4,
                    )
                    for tt in range(TG):
                        t = s * TG + tt
                        mt = mp.tile([P, C, P], F8, tag="mt")
                        nc.sync.dma_start(
                            out=mt[:],
                            in_=d_mm.ap()[t].rearrange("e (k d) -> e k d", d=P),
                        )
                        acc = psA.tile([P, NW], F32, tag="acc")
                        for k in range(C):
                            ysl = (
                                y[:, tt * cA + k, 0:NW]
                                if k < cA
                                else y[:, TG * cA + tt * cB + (k - cA), 0:NW]
                            )
                            nc.tensor.matmul(
                                acc[:],
                                lhsT=mt[:, k, :],
                                rhs=ysl,
                                start=(k == 0),
                                stop=(k == C - 1),
                            )
                        nm = wp.tile([P, NW], BF, tag="nm")
                        nc.vector.tensor_copy(out=nm[:], in_=acc[:])
                        nfc = (NW + P - 1) // P
                        for fc in range(nfc):
                            w_fc = min(P, NW - fc * P)
                            pt = psB.tile([P, P], BF, tag="tp")
                            nc.tensor.transpose(
                                pt[:w_fc, :],
                                nm[:, fc * P : fc * P + w_fc],
                                ident[:],
                            )
                            nc.vector.tensor_copy(
                                out=outT[:w_fc, fc, t * P : (t + 1) * P],
                                in_=pt[:w_fc, :],
                            )
                        if bounce_dram is not None:
                            if bounce_dram is bounce_s:
                                nc.sync.dma_start(
                                    out=bounce_dram.ap()[t * P : (t + 1) * P, 0:NW],
                                    in_=nm[:],
                                )
                            else:
                                nm8 = wp.tile([P, NW], F8, tag="nm8")
                                nc.vector.tensor_copy(out=nm8[:], in_=acc[:])
                                nc.sync.dma_start(
                                    out=bounce_dram.ap()[t * P : (t + 1) * P, 0:NW],
                                    in_=nm8[:],
                                )

            def dense(l_idx, wt, kc, inT0, h_out, last=False):
                """h_out = relu(Tx0@W'0 + Tx1@W'1 + P2@W'2 + b), feature-major;
                also write node-major tiles to bounce (fp8) or bounce16."""
                terms = [(inT0, 0), (t1T, 1), (p2T, 2)]
                for t in range(NT):
                    nm = wp.tile([P, 256], BF if last else F8, tag="nmd")
                    for fo in range(2):
                        pd = psC.tile([P, P], F32, tag="pd")
                        n_mm = len(terms) * kc
                        i_mm = 0
                        for inT, term in terms:
                            for ki in range(kc):
                                nc.tensor.matmul(
                                    pd[:],
                                    lhsT=w_sb[wt][:, term, ki, fo * P : (fo + 1) * P],
                                    rhs=inT[:, ki, t * P : (t + 1) * P],
                                    start=(i_mm == 0),
                                    stop=(i_mm == n_mm - 1),
                                )
                                i_mm += 1
                        nc.scalar.activation(
                            h_out[:, fo, t * P : (t + 1) * P],
                            pd[:],
                            mybir.ActivationFunctionType.Relu,
                            bias=bvec_sb[:, l_idx, fo : fo + 1],
                        )
                        pt = psB.tile([P, P], BF, tag="tp")
                        nc.tensor.transpose(
                            pt[:], h_out[:, fo, t * P : (t + 1) * P], ident[:]
                        )
                        nc.vector.tensor_copy(
                            out=nm[:, fo * P : (fo + 1) * P], in_=pt[:]
                        )
                    nc.sync.dma_start(
                        out=(bounce16 if last else bounce).ap()[
                            t * P : (t + 1) * P, :
                        ],
                        in_=nm[:],
                    )

            def allgather(src, dst):
                nc.gpsimd.collective_compute(
                    "AllGather",
                    mybir.AluOpType.bypass,
                    replica_groups=rg,
                    ins=[src.ap().opt()],
                    outs=[dst.ap().opt()],
                )

            # ================= layer 1 (input x, width 64->200) =================
            prop(d_xg, P, FIN, t1T, bounce_s, BF)
            allgather(bounce_s, t1g_s)
            prop(t1g_s, P, FIN, p2T, None, BF)
            dense(0, "w1", 1, hT[0], hT[1])
            allgather(bounce, hg)

            # ================= layers 2..4 =================
            cur = 1
            for li, wt in ((1, "w2"), (2, "w3"), (3, "w4")):
                prop(hg, 256, H, t1T, bounce, F8)
                allgather(bounce, t1g)
                prop(t1g, 256, H, p2T, None, F8)
                dense(li, wt, 2, hT[cur], hT[1 - cur], last=(li == 3))
                cur = 1 - cur
                if li < 3:
                    allgather(bounce, hg)

            if True:
                # ================= pooling + fc + log_softmax =================
                red = {}
                for typ in range(2):  # 0: mean(sum), 1: max
                    rT = bigp.tile([P, 2, GPC_OUT], F32, tag=f"red{typ}")
                    red[typ] = rT
                    for call in range(n_calls):
                        py = yp.tile([P, 2, POOL_GPC * GW], BF, tag="poolY")
                        base = (typ * n_calls + call) * (POOL_GPC * GW // 16)
                        nc.gpsimd.dma_gather(
                            out_ap=py[:],
                            in_ap=bounce16.ap(),
                            idxs_ap=pidx_sb[:, base : base + POOL_GPC * GW // 16],
                            num_idxs=POOL_GPC * GW,
                            num_idxs_reg=POOL_GPC * GW,
                            elem_size=256,
                            transpose=True,
                            single_packet=False,
                        )
                        for gg in range(POOL_GPC):
                            gcol = call * POOL_GPC + gg
                            for fc in range(2):
                                nc.vector.tensor_reduce(
                                    out=rT[:, fc, gcol : gcol + 1],
                                    in_=py[:, fc, gg * GW : (gg + 1) * GW],
                                    axis=mybir.AxisListType.X,
                                    op=mybir.AluOpType.add if typ == 0 else mybir.AluOpType.max,
                                )
                # mean scale + cast to bf16 lhsT tiles
                pool_bf = bigp.tile([P, 4, GPC_OUT], BF, tag="poolbf")
                for fc in range(2):
                    nc.vector.tensor_tensor(
                        out=pool_bf[:, fc, :],
                        in0=red[0][:, fc, :],
                        in1=cntr_sb[:, :],
                        op=mybir.AluOpType.mult,
                    )
                    nc.vector.tensor_copy(out=pool_bf[:, 2 + fc, :], in_=red[1][:, fc, :])
                pfc = psD.tile([GPC_OUT, 2], F32, tag="pfc")
                for j in range(4):
                    nc.tensor.matmul(
                        pfc[:],
                        lhsT=pool_bf[:, j, :],
                        rhs=fcw_sb[:, j, :],
                        start=(j == 0),
                        stop=(j == 3),
                    )
                z = wp.tile([GPC_OUT, 2], F32, tag="z")
                nc.vector.tensor_tensor(
                    out=z[:], in0=pfc[:], in1=fcb_sb[:, :],
                    op=mybir.AluOpType.add,
                )
                mx = wp.tile([GPC_OUT, 1], F32, tag="mx")
                nc.vector.tensor_reduce(
                    out=mx[:], in_=z[:], axis=mybir.AxisListType.X, op=mybir.AluOpType.max
                )
                zm = wp.tile([GPC_OUT, 2], F32, tag="zm")
                nc.vector.tensor_scalar(
                    zm[:], z[:], mx[:], None, mybir.AluOpType.subtract
                )
                ez = wp.tile([GPC_OUT, 2], F32, tag="ez")
                nc.scalar.activation(ez[:], zm[:], mybir.ActivationFunctionType.Exp)
                sz = wp.tile([GPC_OUT, 1], F32, tag="sz")
                nc.vector.tensor_reduce(
                    out=sz[:], in_=ez[:], axis=mybir.AxisListType.X, op=mybir.AluOpType.add
                )
                lz = wp.tile([GPC_OUT, 1], F32, tag="lz")
                nc.scalar.activation(lz[:], sz[:], mybir.ActivationFunctionType.Ln)
                oz = wp.tile([GPC_OUT, 2], F32, tag="oz")
                nc.vector.tensor_scalar(
                    oz[:], zm[:], lz[:], None, mybir.AluOpType.subtract
                )
                nc.sync.dma_start(out=d_out.ap(), in_=oz[:])

    nc.finalize()
    return nc


def kernel(**inputs):
    x = np.asarray(inputs["x"], np.float32)
    edge_index = np.asarray(inputs["edge_index"])
    batch = np.asarray(inputs["batch"])
    lmax = np.asarray(inputs["lmax"], np.float32)

    pp = _preprocess(x, edge_index, batch, lmax)
    wts = _pack_weights(
        np.asarray(inputs["W1"], np.float32), np.asarray(inputs["W2"], np.float32),
        np.asarray(inputs["W3"], np.float32), np.asarray(inputs["W4"], np.float32),
        np.asarray(inputs["b1"], np.float32), np.asarray(inputs["b2"], np.float32),
        np.asarray(inputs["b3"], np.float32), np.asarray(inputs["b4"], np.float32),
        np.asarray(inputs["fc_w"], np.float32), np.asarray(inputs["fc_b"], np.float32),
    )

    key = (pp["NL"], pp["NT"], pp["C"], pp["cA"], pp["cB"], pp["GW"])
    if key not in _cache:
        _cache[key] = _build(*key)
    nc = _cache[key]

    shared = dict(
        xg=pp["xg"],
        w1=wts["w1"], w2=wts["w2"], w3=wts["w3"], w4=wts["w4"],
        bvec=wts["bvec"], fcw=wts["fcw"], fcb=wts["fcb"],
    )
    in_maps = [
        dict(
            shared,
            xloc=pp["xloc"][c], idx=pp["idx"][c], mm=pp["mm"][c],
            pidx=pp["pidx"][c], cntr=pp["cntr"][c],
        )
        for c in range(NCORES)
    ]
    trace = bool(int(__import__("os").environ.get("KERNEL_TRACE", "0")))
    res = run_bass_kernel_spmd(nc, in_maps, list(range(NCORES)), trace=trace)
    if trace:
        kernel.last_exec_time_ns = res.exec_time_ns
        kernel.last_results = res
    out = np.concatenate([res.results[c]["out"] for c in range(NCORES)], axis=0)
    return out.astype(np.float32)


kernel.last_exec_time_ns = None

